# revision 31
# baseline (speedup 1.0000x reference)
"""TRN2 Bass kernel for causal multi-head attention with RoPE.

Problem: B=2, S=2048, HID=2048, NH=16, HD=128 (fp32 reference).
Sharding: 8 cores = 2 (batch) x 4 (head-groups of 4 heads).
Each core computes q/k/v projections for its 4 heads (column-parallel),
RoPE, causal attention, and a row-parallel partial o_proj; the host sums
the 4 partials per batch in f32.

All-bf16 matmul datapath (fp32 PSUM accumulation), ~380us/core:
  - bf16 halves DMA + SBUF traffic, enables FWL on LDWEIGHTS, and avoids
    the fp32r quarter-rate penalty on trimmed (N<256) diagonal tiles.
  - Q^T/K^T/V stay resident in SBUF (no DRAM spill/reload).
  - output written bf16 (tolerance 2e-2; measured rel err ~4e-3).

Per-core schedule (PE stream ~316us of 22GFLOP at 78.6TF/s peak):
  warmup: ~30 dummy matmuls bridge the initial x DMA, lifting the PE HAM
          clock gate (cold 1.2GHz -> warm 2.4GHz) before real work.
  Q then K projections, seq-chunk-outer / head-inner (matches ~6us/chunk
          HBM delivery vs 13.6us/chunk consumption; x streams on both
          HWDGE rings in quarter-chunks, weights on the SWDGE ring).
          RoPE is fused into the PSUM eviction: two partition-shifted
          DVE reads of PSUM (legal only on a PSUM operand) + cos mul +
          combine add, written bf16 straight into qall/kall.
  V projection last (N=512 pass): hides the K RoPE eviction tail; the
          last 4 of 16 tiles are woven between attention chunk-0 heads
          (per-chunk vsb tiles keep the dependency granularity fine).
  attention, chunk-outer: per (chunk, head): scores^T = K_tile^T @ Q
          with causal tile skipping and N-trimmed diagonal tiles, exp on
          ACT (scale fused) into bf16, 0/1 causal mask mul on DVE (diag
          tiles only), P@V + all-ones-[128,128] softmax-sum matmuls
          accumulate in PSUM (M=128: single-col-group matmuls pay ~85ns
          on both transitions; broadcast sum rows make the normalize a
          full-width rcp + one DVE mul, no partition_broadcast). o_proj for chunk c-1 is emitted during
          chunk c so its matmuls fill the exp-latency bubbles; evictions
          alternate DVE/ACT and the final writes alternate both DMA
          rings.
PSUM (8 banks): QK phase ppq 6 + pg 2; attention sps 2 + ops 4 + pg 2
(pg is shared by warmup/V/o_proj via one tag; same-bank PE-write +
DVE-read is fatal on TRN2, so softmax sums keep their own banks).
"""
import os
import sys

if "/opt/trn_rl_repo" not in sys.path:
    sys.path.insert(0, "/opt/trn_rl_repo")

import numpy as np
import ml_dtypes

import concourse.bass as bass
import concourse.mybir as mybir
import concourse.tile as tile
from concourse import bacc
from concourse.bass_utils import run_bass_kernel_spmd
from contextlib import ExitStack

P = 128
B, S, HID, NH = 2, 2048, 2048, 16
HD = HID // NH              # 128
H = 4                       # heads per core
DPC = H * HD                # 512 dims per core
KO = HID // P               # 16 contraction chunks
SC = S // 512               # 4 seq chunks of 512
ST = S // P                 # 16 seq tiles of 128
SCALE = 1.0 / float(np.sqrt(HD))

f32 = mybir.dt.float32
bf16 = mybir.dt.bfloat16

_CACHED_NC = None


def build_nc():
    AF = mybir.ActivationFunctionType
    nc = bacc.Bacc(None, target_bir_lowering=False)

    xt = nc.declare_dram_parameter("xt", [P, SC, KO, 512], bf16, isOutput=False)
    wq = nc.declare_dram_parameter("wq", [H, P, KO, HD], bf16, isOutput=False)
    wk = nc.declare_dram_parameter("wk", [H, P, KO, HD], bf16, isOutput=False)
    wv = nc.declare_dram_parameter("wv", [P, KO, DPC], bf16, isOutput=False)
    wo = nc.declare_dram_parameter("wo", [P, H, HID], bf16, isOutput=False)
    cosf = nc.declare_dram_parameter("cosf", [P, S], bf16, isOutput=False)
    sinf = nc.declare_dram_parameter("sinf", [P, S], bf16, isOutput=False)
    bmask = nc.declare_dram_parameter("bmask", [P, H, 512], bf16, isOutput=False)
    out_p = nc.declare_dram_parameter("out_p", [S, HID], bf16, isOutput=True)

    out3 = out_p.rearrange("(st p) n -> p st n", p=P)

    with tile.TileContext(nc) as tc:
        with ExitStack() as top:
            vpool = top.enter_context(tc.tile_pool(name="vpool", bufs=1))
            const = top.enter_context(tc.tile_pool(name="const", bufs=1))
            qkpool = top.enter_context(tc.tile_pool(name="qkall", bufs=1))
            ppool = top.enter_context(tc.tile_pool(name="ppool", bufs=6))
            stage = top.enter_context(tc.tile_pool(name="stage", bufs=3))
            aopool = top.enter_context(tc.tile_pool(name="ao", bufs=1))
            ost = top.enter_context(tc.tile_pool(name="ost", bufs=4))
            # 2-bank PSUM pool shared (same tag) by warm-up, V projection and
            # o_proj accumulators -- their lifetimes are disjoint
            pg = top.enter_context(tc.tile_pool(name="pg", bufs=2, space="PSUM"))

            # V tiles split per seq-chunk so chunk-0 attention depends only
            # on the first 4 V evictions (V tiles 12-15 interleave into it)
            vsb_c = [
                vpool.tile([P, 4, H, HD], bf16, name=f"vsb{cc}")
                for cc in range(SC)
            ]
            qall = qkpool.tile([P, H, S], bf16, name="qall")
            kall = qkpool.tile([P, H, S], bf16, name="kall")
            # only chunk 0's attn-out tile preallocates; 1-3 open after the
            # x pool is released (their first writes happen later)
            aot_c = {0: aopool.tile([P, H, 512], bf16, tag="aot0", name="aot0")}

            zb = const.tile([P, 1], f32)
            # full-width all-ones stationary: the softmax-sum matmul runs at
            # M=128 (every output row = the sum) -- M=1 col-group matmuls pay
            # ~+85ns on both neighboring transitions, and the broadcast rows
            # make the gpsimd partition_broadcast unnecessary
            ones_mat = const.tile([P, P], bf16)
            bmt = const.tile([P, H, 512], bf16)
            nc.vector.memset(zb[:], 0.0)
            nc.vector.memset(ones_mat[:], 1.0)
            # warm-up reads a zeroed slice of qall (overwritten later by RoPE)
            nc.vector.memset(qall[:, 0, 0:512], 0.0)

            # ---------------- Phase P: projections (Q, K, then V) ----------
            # Q/K first: they gate attention start and need only x chunk 0
            # plus head-0 weights to begin; chunk-outer order matches the
            # ~6us/chunk HBM delivery rate against 13.6us/chunk consumption.
            # V last: it hides the K RoPE eviction tail under dependency-free
            # PE work.
            pctx = ExitStack()
            xpool = pctx.enter_context(tc.tile_pool(name="xp", bufs=1))
            wvp = pctx.enter_context(tc.tile_pool(name="wvp", bufs=1))
            xs = xpool.tile([P, SC, KO, 512], bf16)
            wvt = wvp.tile([P, KO, DPC], bf16)

            with ExitStack() as ctx:
                ppq = ctx.enter_context(tc.tile_pool(name="ppq", bufs=6, space="PSUM"))
                wpool = ctx.enter_context(tc.tile_pool(name="wqk", bufs=1))
                cspool = ctx.enter_context(tc.tile_pool(name="cs", bufs=1))
                rtmp = ctx.enter_context(tc.tile_pool(name="rt", bufs=2))

                wqs = wpool.tile([P, H, KO, HD], bf16, name="wqs")
                wks = wpool.tile([P, H, KO, HD], bf16, name="wks")
                cosT = cspool.tile([P, S], bf16)
                sinT = cspool.tile([P, S], bf16)

                # x chunks stream on the two HWDGE rings ahead of everything
                # else; q/k weights arrive per-head via the idle SWDGE ring;
                # rope tables interleave after chunk 0; wv/bmask last.
                for h in range(H):
                    nc.gpsimd.dma_start(wqs[:, h], wq[h])
                for h in range(H):
                    nc.gpsimd.dma_start(wks[:, h], wk[h])
                nc.gpsimd.dma_start(bmt[:], bmask[:])
                # chunk 0 arrives column-split: the first Q group starts
                # on the low 256 columns (1MB) while the high half streams
                nc.sync.dma_start(xs[:, 0, 0:8, 0:256], xt[:, 0, 0:8, 0:256])
                nc.scalar.dma_start(xs[:, 0, 8:16, 0:256], xt[:, 0, 8:16, 0:256])
                nc.sync.dma_start(xs[:, 0, 0:8, 256:512], xt[:, 0, 0:8, 256:512])
                nc.scalar.dma_start(xs[:, 0, 8:16, 256:512], xt[:, 0, 8:16, 256:512])
                for sc in range(1, SC):
                    nc.sync.dma_start(xs[:, sc, 0:4], xt[:, sc, 0:4])
                    nc.scalar.dma_start(xs[:, sc, 8:12], xt[:, sc, 8:12])
                    nc.sync.dma_start(xs[:, sc, 4:8], xt[:, sc, 4:8])
                    nc.scalar.dma_start(xs[:, sc, 12:16], xt[:, sc, 12:16])
                    # rope tables after chunk 1: x chunks must outrun the
                    # first Q sweep; the RoPE eviction they feed has ~20us
                    # of PSUM-pool slack and tolerates their later arrival
                    if sc == 1:
                        nc.sync.dma_start(cosT[:], cosf[:])
                        nc.scalar.dma_start(sinT[:], sinf[:])
                nc.sync.dma_start(wvt[:, :, 0:256], wv[:, :, 0:256])
                nc.scalar.dma_start(wvt[:, :, 256:512], wv[:, :, 256:512])

                # HAM warm-up: dummy matmuls bridge until the first x chunk
                # lands, lifting the PE clock gate (cold 1.2 -> warm 2.4 GHz).
                wps = pg.tile([P, 512], f32, tag="gemm")
                for i in range(24):
                    nc.tensor.matmul(wps[:], qall[:, 0, 0:P], qall[:, 0, 0:512],
                                     start=(i == 0), stop=(i == 23))

                # Q/K projections with RoPE fused into the PSUM eviction.
                # cos table has duplicated halves; sin is signed (-sin rows
                # 0:64, +sin rows 64:128) so the combine is one add.
                for ws, dst in ((wqs, qall), (wks, kall)):
                    for sc in range(SC):
                        ssl = slice(sc * 512, (sc + 1) * 512)
                        for h in range(H):
                            ps = ppq.tile([P, 512], f32, tag="proj")
                            if ws is wqs and sc == 0 and h == 0:
                                # gated only on the low column half (1MB):
                                # two 16-MM accumulations; the second half's
                                # bank-wide has_written clear preserves the
                                # first half's finished data
                                for cf in range(2):
                                    csl = slice(cf * 256, (cf + 1) * 256)
                                    for ko in range(KO):
                                        nc.tensor.matmul(
                                            ps[:, csl],
                                            ws[:, h, ko],
                                            xs[:, sc, ko, csl],
                                            start=(ko == 0),
                                            stop=(ko == KO - 1),
                                            skip_group_check=True,
                                        )
                            else:
                                for ko in range(KO):
                                    nc.tensor.matmul(
                                        ps[:],
                                        ws[:, h, ko],
                                        xs[:, sc, ko],
                                        start=(ko == 0),
                                        stop=(ko == KO - 1),
                                    )
                            # partition-shifted reads are legal only with a
                            # PSUM operand, so the two rotate half-ops read
                            # ps directly; combine writes bf16 to SBUF.
                            t0 = rtmp.tile([P, 512], f32, tag="t0")
                            spt = rtmp.tile([P, 512], f32, tag="spl")
                            nc.vector.tensor_mul(t0[0:64], ps[64:128], sinT[0:64, ssl])
                            nc.vector.tensor_mul(t0[64:128], ps[0:64], sinT[64:128, ssl])
                            nc.vector.tensor_mul(spt[:], ps[:], cosT[:, ssl])
                            nc.vector.tensor_add(dst[:, h, ssl], spt[:], t0[:])

            # QK psum/weight/rope pools released; attention PSUM pools open
            spsum = top.enter_context(tc.tile_pool(name="sps", bufs=2, space="PSUM"))
            opsum = top.enter_context(tc.tile_pool(name="ops", bufs=2, space="PSUM"))

            def emit_vproj(st):
                sc, sub = st // 4, st % 4
                ps = pg.tile([P, 512], f32, tag="gemm")
                for ko in range(KO):
                    nc.tensor.matmul(
                        ps[:],
                        xs[:, sc, ko, sub * P:(sub + 1) * P],
                        wvt[:, ko],
                        start=(ko == 0),
                        stop=(ko == KO - 1),
                    )
                nc.vector.tensor_copy(
                    vsb_c[sc][:, sub], ps.rearrange("p (h d) -> p h d", h=H)
                )

            # V projection tiles 0-11; 12-15 interleave into attention c0
            for st in range(12):
                emit_vproj(st)

            # ------------- Phase A: attention + interleaved o_proj ---------
            def emit_oproj(cc):
                for st4 in range(4):
                    st = cc * 4 + st4
                    for nch in range(4):
                        pso = pg.tile([P, 512], f32, tag="gemm", name="pso")
                        for dc in range(H):
                            nc.tensor.matmul(
                                pso[:],
                                aot_c[cc][:, dc, st4 * P:(st4 + 1) * P],
                                wot[:, dc, nch * 512:(nch + 1) * 512],
                                start=(dc == 0),
                                stop=(dc == H - 1),
                            )
                        ob = ost.tile([P, 512], bf16, tag="ob", name="ob")
                        # alternate eviction engines + (last chunk) DMA
                        # rings so the post-compute drain is not serial
                        # on a single queue
                        if nch % 2 == 0:
                            nc.vector.tensor_copy(ob[:], pso[:])
                        else:
                            nc.scalar.activation(ob[:], pso[:], AF.Copy)
                        deng = nc.scalar if (cc == SC - 1 and nch % 2 == 1) \
                            else nc.sync
                        deng.dma_start(
                            out3[:, st, nch * 512:(nch + 1) * 512], ob[:]
                        )

            for c in range(SC):
                nt = 4 * (c + 1)
                for h in range(H):
                    # attn_outT accumulator [d, sq] and softmax sums [1, sq]
                    ob_ps = opsum.tile([P, 512], f32, tag="obp", name="obp")
                    sm_ps = opsum.tile([P, 512], f32, tag="smp", name="smp")
                    # diagonal tiles first: their exp+mask latency hides
                    # behind the dense unmasked tail of this head and the
                    # previous head's stream
                    t_order = list(range(4 * c, nt)) + list(range(0, 4 * c))
                    for ti, t in enumerate(t_order):
                        r = t - 4 * c
                        off = P * max(r, 0)
                        ps = spsum.tile([P, 512], f32, tag="s")
                        nc.tensor.matmul(
                            ps[:, off:512],
                            kall[:, h, t * P:(t + 1) * P],
                            qall[:, h, c * 512 + off:(c + 1) * 512],
                            start=True,
                            stop=True,
                        )
                        pt = ppool.tile([P, 512], bf16, tag="pt")
                        nc.scalar.activation(
                            pt[:, off:512], ps[:, off:512], AF.Exp,
                            bias=zb[:], scale=SCALE,
                        )
                        if r >= 0:
                            nc.vector.tensor_mul(
                                pt[:, off:512], pt[:, off:512], bmt[:, r, off:512]
                            )
                        # P@V with V stationary; output is attn_outT [d, sq]
                        nc.tensor.matmul(
                            ob_ps[:, off:512],
                            vsb_c[t // 4][:, t % 4, h],
                            pt[:, off:512],
                            start=(ti == 0),
                            stop=(ti == nt - 1),
                        )
                        nc.tensor.matmul(
                            sm_ps[:, off:512],
                            ones_mat[:],
                            pt[:, off:512],
                            start=(ti == 0),
                            stop=(ti == nt - 1),
                        )
                    # normalize: full-width rcp of the broadcast sums ->
                    # single DVE multiply into attn_outT SBUF (bf16)
                    rcp = stage.tile([P, 512], f32, tag="bc")
                    nc.vector.reciprocal_approx_fast(rcp[:], sm_ps[:])
                    nc.vector.tensor_mul(aot_c[c][:, h], ob_ps[:], rcp[:])
                    # V tiles 12-15 woven between chunk-0 heads: they fill
                    # the exp/normalize latency bubbles (c0 has no o_proj
                    # filler yet)
                    if c == 0:
                        emit_vproj(12 + h)

                if c == 0:
                    # x / V-weight resources die here; o_proj weights and the
                    # remaining attn-out tiles take the freed space
                    pctx.close()
                    wopool = top.enter_context(tc.tile_pool(name="wop", bufs=1))
                    wot = wopool.tile([P, H, HID], bf16)
                    nc.sync.dma_start(wot[:], wo[:])
                    for cc in range(1, SC):
                        aot_c[cc] = wopool.tile(
                            [P, H, 512], bf16, tag=f"aot{cc}", name=f"aot{cc}")
                # o_proj deferred by one chunk: its aot inputs are then
                # guaranteed ready, so the PE stream never stalls on the
                # normalize tail
                if c > 0:
                    emit_oproj(c - 1)
            emit_oproj(SC - 1)

    nc.compile()
    return nc


def _host_prep(hidden_states, position_ids, Wq, Wk, Wv, Wo):
    """Build the 8 per-core input maps (bf16 device layouts)."""
    inv_freq = 1.0 / (10000.0 ** (np.arange(0, HD, 2, dtype=np.float32) / HD))
    t = np.arange(S, dtype=np.float32)
    freqs = np.outer(t, inv_freq).astype(np.float32)  # [S, 64]

    bm = np.empty((P, H, 512), dtype=np.float32)
    i = np.arange(P)[:, None, None]
    r = np.arange(H)[None, :, None]
    j = np.arange(512)[None, None, :]
    bm[:] = np.where(i + P * r <= j, 1.0, 0.0)
    bm = bm.astype(ml_dtypes.bfloat16)

    in_maps = []
    per_batch = []
    for b in range(B):
        xT = np.ascontiguousarray(hidden_states[b].T)  # [HID, S]
        xt_sw = np.ascontiguousarray(
            xT.reshape(KO, P, SC, 512).transpose(1, 2, 0, 3)
        ).astype(ml_dtypes.bfloat16)  # [P, SC, KO, 512]
        fp = freqs[position_ids[b]]  # [S, 64]
        ch = np.cos(fp).T            # [64, S]
        sh = np.sin(fp).T
        cosf = np.ascontiguousarray(
            np.concatenate([ch, ch], axis=0)).astype(ml_dtypes.bfloat16)   # [128, S]
        sinf = np.ascontiguousarray(
            np.concatenate([-sh, sh], axis=0)).astype(ml_dtypes.bfloat16)  # signed
        per_batch.append((xt_sw, cosf, sinf))

    for core in range(8):
        b, hg = core // 4, core % 4
        sl = slice(hg * DPC, (hg + 1) * DPC)
        xt_sw, cosf, sinf = per_batch[b]
        wq_sw = np.ascontiguousarray(
            Wq[sl].T.reshape(KO, P, H, HD).transpose(2, 1, 0, 3)
        ).astype(ml_dtypes.bfloat16)  # [H, P, KO, HD]
        wk_sw = np.ascontiguousarray(
            Wk[sl].T.reshape(KO, P, H, HD).transpose(2, 1, 0, 3)
        ).astype(ml_dtypes.bfloat16)
        wv_sw = np.ascontiguousarray(
            Wv[sl].T.reshape(KO, P, DPC).transpose(1, 0, 2)
        ).astype(ml_dtypes.bfloat16)  # [P, KO, DPC]
        wo_sw = np.ascontiguousarray(
            Wo[:, sl].T.reshape(H, HD, HID).transpose(1, 0, 2)
        ).astype(ml_dtypes.bfloat16)  # [P, H, HID]
        in_maps.append({
            "xt": xt_sw, "wq": wq_sw, "wk": wk_sw, "wv": wv_sw, "wo": wo_sw,
            "cosf": cosf, "sinf": sinf, "bmask": bm,
        })
    return in_maps


def kernel(hidden_states, attention_mask, position_ids, Wq, Wk, Wv, Wo,
           _trace=False, _trace_kwargs=None):
    global _CACHED_NC
    hidden_states = np.asarray(hidden_states, dtype=np.float32)
    position_ids = np.asarray(position_ids)
    Wq, Wk, Wv, Wo = (np.asarray(w, dtype=np.float32) for w in (Wq, Wk, Wv, Wo))

    if _CACHED_NC is None:
        _CACHED_NC = build_nc()
    nc = _CACHED_NC

    in_maps = _host_prep(hidden_states, position_ids, Wq, Wk, Wv, Wo)
    res = run_bass_kernel_spmd(
        nc, in_maps, list(range(8)), trace=_trace, **(_trace_kwargs or {})
    )

    out = np.empty((B, S, HID), dtype=np.float32)
    for b in range(B):
        acc = res.results[b * 4]["out_p"].astype(np.float32)
        for hg in range(1, 4):
            acc = acc + res.results[b * 4 + hg]["out_p"].astype(np.float32)
        out[b] = acc
    if _trace:
        return out, res
    return out


# revision 32
# speedup vs baseline: 1.0389x; 1.0389x over previous
"""TRN2 Bass kernel for causal multi-head attention with RoPE.

Problem: B=2, S=2048, HID=2048, NH=16, HD=128 (fp32 reference).
Sharding: 8 cores = 2 (batch) x 4 (head-groups of 4 heads).
Each core computes q/k/v projections for its 4 heads (column-parallel),
RoPE, causal attention, and a row-parallel partial o_proj; the host sums
the 4 partials per batch in f32.

All-bf16 matmul datapath (fp32 PSUM accumulation), ~380us/core:
  - bf16 halves DMA + SBUF traffic, enables FWL on LDWEIGHTS, and avoids
    the fp32r quarter-rate penalty on trimmed (N<256) diagonal tiles.
  - Q^T/K^T/V stay resident in SBUF (no DRAM spill/reload).
  - output written bf16 (tolerance 2e-2; measured rel err ~4e-3).

Per-core schedule (PE stream ~316us of 22GFLOP at 78.6TF/s peak):
  warmup: ~30 dummy matmuls bridge the initial x DMA, lifting the PE HAM
          clock gate (cold 1.2GHz -> warm 2.4GHz) before real work.
  Q then K projections, seq-chunk-outer / head-inner (matches ~6us/chunk
          HBM delivery vs 13.6us/chunk consumption; x streams on both
          HWDGE rings in quarter-chunks, weights on the SWDGE ring).
          RoPE is fused into the PSUM eviction: two partition-shifted
          DVE reads of PSUM (legal only on a PSUM operand) + cos mul +
          combine add, written bf16 straight into qall/kall.
  V projection last (N=512 pass): hides the K RoPE eviction tail; the
          last 4 of 16 tiles are woven between attention chunk-0 heads
          (per-chunk vsb tiles keep the dependency granularity fine).
  attention, chunk-outer: per (chunk, head): scores^T = K_tile^T @ Q
          with causal tile skipping and N-trimmed diagonal tiles, exp on
          ACT (scale fused) into bf16, 0/1 causal mask mul on DVE (diag
          tiles only), P@V + all-ones-[128,128] softmax-sum matmuls
          accumulate in PSUM (M=128: single-col-group matmuls pay ~85ns
          on both transitions; broadcast sum rows make the normalize a
          full-width rcp + one DVE mul, no partition_broadcast). o_proj for chunk c-1 is emitted during
          chunk c so its matmuls fill the exp-latency bubbles; evictions
          alternate DVE/ACT and the final writes alternate both DMA
          rings.
PSUM (8 banks): QK phase ppq 6 + pg 2; attention sps 2 + ops 4 + pg 2
(pg is shared by warmup/V/o_proj via one tag; same-bank PE-write +
DVE-read is fatal on TRN2, so softmax sums keep their own banks).
"""
import os
import sys

if "/opt/trn_rl_repo" not in sys.path:
    sys.path.insert(0, "/opt/trn_rl_repo")

import numpy as np
import ml_dtypes

import concourse.bass as bass
import concourse.mybir as mybir
import concourse.tile as tile
from concourse import bacc
from concourse.bass_utils import run_bass_kernel_spmd
from contextlib import ExitStack

P = 128
B, S, HID, NH = 2, 2048, 2048, 16
HD = HID // NH              # 128
H = 4                       # heads per core
DPC = H * HD                # 512 dims per core
KO = HID // P               # 16 contraction chunks
SC = S // 512               # 4 seq chunks of 512
ST = S // P                 # 16 seq tiles of 128
SCALE = 1.0 / float(np.sqrt(HD))

f32 = mybir.dt.float32
bf16 = mybir.dt.bfloat16

_CACHED_NC = None


def build_nc():
    AF = mybir.ActivationFunctionType
    nc = bacc.Bacc(None, target_bir_lowering=False)

    xt = nc.declare_dram_parameter("xt", [P, SC, KO, 512], bf16, isOutput=False)
    wq = nc.declare_dram_parameter("wq", [H, P, KO, HD], bf16, isOutput=False)
    wk = nc.declare_dram_parameter("wk", [H, P, KO, HD], bf16, isOutput=False)
    wv = nc.declare_dram_parameter("wv", [P, KO, DPC], bf16, isOutput=False)
    wo = nc.declare_dram_parameter("wo", [P, H, HID], bf16, isOutput=False)
    cosf = nc.declare_dram_parameter("cosf", [P, S], bf16, isOutput=False)
    sinf = nc.declare_dram_parameter("sinf", [P, S], bf16, isOutput=False)
    bmask = nc.declare_dram_parameter("bmask", [P, H, 512], bf16, isOutput=False)
    out_p = nc.declare_dram_parameter("out_p", [S, HID], bf16, isOutput=True)

    out3 = out_p.rearrange("(st p) n -> p st n", p=P)

    with tile.TileContext(nc) as tc:
        with ExitStack() as top:
            vpool = top.enter_context(tc.tile_pool(name="vpool", bufs=1))
            const = top.enter_context(tc.tile_pool(name="const", bufs=1))
            qkpool = top.enter_context(tc.tile_pool(name="qkall", bufs=1))
            ppool = top.enter_context(tc.tile_pool(name="ppool", bufs=6))
            stage = top.enter_context(tc.tile_pool(name="stage", bufs=3))
            aopool = top.enter_context(tc.tile_pool(name="ao", bufs=1))
            ost = top.enter_context(tc.tile_pool(name="ost", bufs=4))
            # 2-bank PSUM pool shared (same tag) by warm-up, V projection and
            # o_proj accumulators -- their lifetimes are disjoint
            pg = top.enter_context(tc.tile_pool(name="pg", bufs=2, space="PSUM"))

            # V tiles split per seq-chunk so chunk-0 attention depends only
            # on the first 4 V evictions (V tiles 12-15 interleave into it)
            vsb_c = [
                vpool.tile([P, 4, H, HD], bf16, name=f"vsb{cc}")
                for cc in range(SC)
            ]
            qall = qkpool.tile([P, H, S], bf16, name="qall")
            kall = qkpool.tile([P, H, S], bf16, name="kall")
            # only chunk 0's attn-out tile preallocates; 1-3 open after the
            # x pool is released (their first writes happen later)
            aot_c = {0: aopool.tile([P, H, 512], bf16, tag="aot0", name="aot0")}

            zb = const.tile([P, 1], f32)
            # full-width all-ones stationary: the softmax-sum matmul runs at
            # M=128 (every output row = the sum) -- M=1 col-group matmuls pay
            # ~+85ns on both neighboring transitions, and the broadcast rows
            # make the gpsimd partition_broadcast unnecessary
            ones_mat = const.tile([P, P], bf16)
            bmt = const.tile([P, H, 512], bf16)
            nc.vector.memset(zb[:], 0.0)
            nc.vector.memset(ones_mat[:], 1.0)
            # warm-up reads a zeroed slice of qall (overwritten later by RoPE)
            nc.vector.memset(qall[:, 0, 0:512], 0.0)

            # ---------------- Phase P: projections (Q, K, then V) ----------
            # Q/K first: they gate attention start and need only x chunk 0
            # plus head-0 weights to begin; chunk-outer order matches the
            # ~6us/chunk HBM delivery rate against 13.6us/chunk consumption.
            # V last: it hides the K RoPE eviction tail under dependency-free
            # PE work.
            pctx = ExitStack()
            xpool = pctx.enter_context(tc.tile_pool(name="xp", bufs=1))
            wvp = pctx.enter_context(tc.tile_pool(name="wvp", bufs=1))
            xs = xpool.tile([P, SC, KO, 512], bf16)
            wvt = wvp.tile([P, KO, DPC], bf16)

            with ExitStack() as ctx:
                ppq = ctx.enter_context(tc.tile_pool(name="ppq", bufs=6, space="PSUM"))
                wpool = ctx.enter_context(tc.tile_pool(name="wqk", bufs=1))
                cspool = ctx.enter_context(tc.tile_pool(name="cs", bufs=1))
                rtmp = ctx.enter_context(tc.tile_pool(name="rt", bufs=2))

                wqs = wpool.tile([P, H, KO, HD], bf16, name="wqs")
                wks = wpool.tile([P, H, KO, HD], bf16, name="wks")
                cosT = cspool.tile([P, S], bf16)
                sinT = cspool.tile([P, S], bf16)

                # x chunks stream on the two HWDGE rings ahead of everything
                # else; q/k weights arrive per-head via the idle SWDGE ring;
                # rope tables interleave after chunk 0; wv/bmask last.
                for h in range(H):
                    nc.gpsimd.dma_start(wqs[:, h], wq[h])
                for h in range(H):
                    nc.gpsimd.dma_start(wks[:, h], wk[h])
                nc.gpsimd.dma_start(bmt[:], bmask[:])
                for sc in range(SC):
                    nc.sync.dma_start(xs[:, sc, 0:4], xt[:, sc, 0:4])
                    nc.scalar.dma_start(xs[:, sc, 8:12], xt[:, sc, 8:12])
                    nc.sync.dma_start(xs[:, sc, 4:8], xt[:, sc, 4:8])
                    nc.scalar.dma_start(xs[:, sc, 12:16], xt[:, sc, 12:16])
                    # rope tables after chunk 1: x chunks must outrun the
                    # first Q sweep; the RoPE eviction they feed has ~20us
                    # of PSUM-pool slack and tolerates their later arrival
                    if sc == 1:
                        nc.sync.dma_start(cosT[:], cosf[:])
                        nc.scalar.dma_start(sinT[:], sinf[:])
                nc.sync.dma_start(wvt[:, :, 0:256], wv[:, :, 0:256])
                nc.scalar.dma_start(wvt[:, :, 256:512], wv[:, :, 256:512])

                # HAM warm-up: dummy matmuls bridge until the first x chunk
                # lands, lifting the PE clock gate (cold 1.2 -> warm 2.4 GHz).
                wps = pg.tile([P, 512], f32, tag="gemm")
                for i in range(30):
                    nc.tensor.matmul(wps[:], qall[:, 0, 0:P], qall[:, 0, 0:512],
                                     start=(i == 0), stop=(i == 29))

                # Q/K projections with RoPE fused into the PSUM eviction.
                # cos table has duplicated halves; sin is signed (-sin rows
                # 0:64, +sin rows 64:128) so the combine is one add.
                for ws, dst in ((wqs, qall), (wks, kall)):
                    for sc in range(SC):
                        ssl = slice(sc * 512, (sc + 1) * 512)
                        for h in range(H):
                            ps = ppq.tile([P, 512], f32, tag="proj")
                            for ko in range(KO):
                                nc.tensor.matmul(
                                    ps[:],
                                    ws[:, h, ko],
                                    xs[:, sc, ko],
                                    start=(ko == 0),
                                    stop=(ko == KO - 1),
                                )
                            # partition-shifted reads are legal only with a
                            # PSUM operand, so the two rotate half-ops read
                            # ps directly; combine writes bf16 to SBUF.
                            t0 = rtmp.tile([P, 512], f32, tag="t0")
                            spt = rtmp.tile([P, 512], f32, tag="spl")
                            nc.vector.tensor_mul(t0[0:64], ps[64:128], sinT[0:64, ssl])
                            nc.vector.tensor_mul(t0[64:128], ps[0:64], sinT[64:128, ssl])
                            nc.vector.tensor_mul(spt[:], ps[:], cosT[:, ssl])
                            nc.vector.tensor_add(dst[:, h, ssl], spt[:], t0[:])

            # QK psum/weight/rope pools released; attention PSUM pools open
            spsum = top.enter_context(tc.tile_pool(name="sps", bufs=2, space="PSUM"))
            opsum = top.enter_context(tc.tile_pool(name="ops", bufs=2, space="PSUM"))

            def emit_vproj(st):
                sc, sub = st // 4, st % 4
                ps = pg.tile([P, 512], f32, tag="gemm")
                for ko in range(KO):
                    nc.tensor.matmul(
                        ps[:],
                        xs[:, sc, ko, sub * P:(sub + 1) * P],
                        wvt[:, ko],
                        start=(ko == 0),
                        stop=(ko == KO - 1),
                    )
                nc.vector.tensor_copy(
                    vsb_c[sc][:, sub], ps.rearrange("p (h d) -> p h d", h=H)
                )

            # V projection tiles 0-11; 12-15 interleave into attention c0
            for st in range(12):
                emit_vproj(st)

            # ------------- Phase A: attention + interleaved o_proj ---------
            def emit_oproj(cc):
                for st4 in range(4):
                    st = cc * 4 + st4
                    for nch in range(4):
                        pso = pg.tile([P, 512], f32, tag="gemm", name="pso")
                        for dc in range(H):
                            nc.tensor.matmul(
                                pso[:],
                                aot_c[cc][:, dc, st4 * P:(st4 + 1) * P],
                                wot[:, dc, nch * 512:(nch + 1) * 512],
                                start=(dc == 0),
                                stop=(dc == H - 1),
                            )
                        ob = ost.tile([P, 512], bf16, tag="ob", name="ob")
                        # alternate eviction engines + (last chunk) DMA
                        # rings so the post-compute drain is not serial
                        # on a single queue
                        if nch % 2 == 0:
                            nc.vector.tensor_copy(ob[:], pso[:])
                        else:
                            nc.scalar.activation(ob[:], pso[:], AF.Copy)
                        deng = nc.scalar if (cc == SC - 1 and nch % 2 == 1) \
                            else nc.sync
                        deng.dma_start(
                            out3[:, st, nch * 512:(nch + 1) * 512], ob[:]
                        )

            for c in range(SC):
                nt = 4 * (c + 1)
                for h in range(H):
                    # attn_outT accumulator [d, sq] and softmax sums [1, sq]
                    ob_ps = opsum.tile([P, 512], f32, tag="obp", name="obp")
                    sm_ps = opsum.tile([P, 512], f32, tag="smp", name="smp")
                    # diagonal tiles first: their exp+mask latency hides
                    # behind the dense unmasked tail of this head and the
                    # previous head's stream
                    t_order = list(range(4 * c, nt)) + list(range(0, 4 * c))
                    for ti, t in enumerate(t_order):
                        r = t - 4 * c
                        off = P * max(r, 0)
                        ps = spsum.tile([P, 512], f32, tag="s")
                        nc.tensor.matmul(
                            ps[:, off:512],
                            kall[:, h, t * P:(t + 1) * P],
                            qall[:, h, c * 512 + off:(c + 1) * 512],
                            start=True,
                            stop=True,
                        )
                        pt = ppool.tile([P, 512], bf16, tag="pt")
                        nc.scalar.activation(
                            pt[:, off:512], ps[:, off:512], AF.Exp,
                            bias=zb[:], scale=SCALE,
                        )
                        if r >= 0:
                            nc.vector.tensor_mul(
                                pt[:, off:512], pt[:, off:512], bmt[:, r, off:512]
                            )
                        # P@V with V stationary; output is attn_outT [d, sq]
                        nc.tensor.matmul(
                            ob_ps[:, off:512],
                            vsb_c[t // 4][:, t % 4, h],
                            pt[:, off:512],
                            start=(ti == 0),
                            stop=(ti == nt - 1),
                        )
                        nc.tensor.matmul(
                            sm_ps[:, off:512],
                            ones_mat[:],
                            pt[:, off:512],
                            start=(ti == 0),
                            stop=(ti == nt - 1),
                        )
                    # normalize: full-width rcp of the broadcast sums ->
                    # single DVE multiply into attn_outT SBUF (bf16)
                    rcp = stage.tile([P, 512], f32, tag="bc")
                    nc.vector.reciprocal_approx_fast(rcp[:], sm_ps[:])
                    nc.vector.tensor_mul(aot_c[c][:, h], ob_ps[:], rcp[:])
                    # V tiles 12-15 woven between chunk-0 heads: they fill
                    # the exp/normalize latency bubbles (c0 has no o_proj
                    # filler yet)
                    if c == 0:
                        emit_vproj(12 + h)

                if c == 0:
                    # x / V-weight resources die here; o_proj weights and the
                    # remaining attn-out tiles take the freed space
                    pctx.close()
                    wopool = top.enter_context(tc.tile_pool(name="wop", bufs=1))
                    wot = wopool.tile([P, H, HID], bf16)
                    nc.sync.dma_start(wot[:], wo[:])
                    for cc in range(1, SC):
                        aot_c[cc] = wopool.tile(
                            [P, H, 512], bf16, tag=f"aot{cc}", name=f"aot{cc}")
                # o_proj deferred by one chunk: its aot inputs are then
                # guaranteed ready, so the PE stream never stalls on the
                # normalize tail
                if c > 0:
                    emit_oproj(c - 1)
            emit_oproj(SC - 1)

    nc.compile()
    return nc


def _host_prep(hidden_states, position_ids, Wq, Wk, Wv, Wo):
    """Build the 8 per-core input maps (bf16 device layouts)."""
    inv_freq = 1.0 / (10000.0 ** (np.arange(0, HD, 2, dtype=np.float32) / HD))
    t = np.arange(S, dtype=np.float32)
    freqs = np.outer(t, inv_freq).astype(np.float32)  # [S, 64]

    bm = np.empty((P, H, 512), dtype=np.float32)
    i = np.arange(P)[:, None, None]
    r = np.arange(H)[None, :, None]
    j = np.arange(512)[None, None, :]
    bm[:] = np.where(i + P * r <= j, 1.0, 0.0)
    bm = bm.astype(ml_dtypes.bfloat16)

    in_maps = []
    per_batch = []
    for b in range(B):
        xT = np.ascontiguousarray(hidden_states[b].T)  # [HID, S]
        xt_sw = np.ascontiguousarray(
            xT.reshape(KO, P, SC, 512).transpose(1, 2, 0, 3)
        ).astype(ml_dtypes.bfloat16)  # [P, SC, KO, 512]
        fp = freqs[position_ids[b]]  # [S, 64]
        ch = np.cos(fp).T            # [64, S]
        sh = np.sin(fp).T
        cosf = np.ascontiguousarray(
            np.concatenate([ch, ch], axis=0)).astype(ml_dtypes.bfloat16)   # [128, S]
        sinf = np.ascontiguousarray(
            np.concatenate([-sh, sh], axis=0)).astype(ml_dtypes.bfloat16)  # signed
        per_batch.append((xt_sw, cosf, sinf))

    for core in range(8):
        b, hg = core // 4, core % 4
        sl = slice(hg * DPC, (hg + 1) * DPC)
        xt_sw, cosf, sinf = per_batch[b]
        wq_sw = np.ascontiguousarray(
            Wq[sl].T.reshape(KO, P, H, HD).transpose(2, 1, 0, 3)
        ).astype(ml_dtypes.bfloat16)  # [H, P, KO, HD]
        wk_sw = np.ascontiguousarray(
            Wk[sl].T.reshape(KO, P, H, HD).transpose(2, 1, 0, 3)
        ).astype(ml_dtypes.bfloat16)
        wv_sw = np.ascontiguousarray(
            Wv[sl].T.reshape(KO, P, DPC).transpose(1, 0, 2)
        ).astype(ml_dtypes.bfloat16)  # [P, KO, DPC]
        wo_sw = np.ascontiguousarray(
            Wo[:, sl].T.reshape(H, HD, HID).transpose(1, 0, 2)
        ).astype(ml_dtypes.bfloat16)  # [P, H, HID]
        in_maps.append({
            "xt": xt_sw, "wq": wq_sw, "wk": wk_sw, "wv": wv_sw, "wo": wo_sw,
            "cosf": cosf, "sinf": sinf, "bmask": bm,
        })
    return in_maps


def kernel(hidden_states, attention_mask, position_ids, Wq, Wk, Wv, Wo,
           _trace=False, _trace_kwargs=None):
    global _CACHED_NC
    hidden_states = np.asarray(hidden_states, dtype=np.float32)
    position_ids = np.asarray(position_ids)
    Wq, Wk, Wv, Wo = (np.asarray(w, dtype=np.float32) for w in (Wq, Wk, Wv, Wo))

    if _CACHED_NC is None:
        _CACHED_NC = build_nc()
    nc = _CACHED_NC

    in_maps = _host_prep(hidden_states, position_ids, Wq, Wk, Wv, Wo)
    res = run_bass_kernel_spmd(
        nc, in_maps, list(range(8)), trace=_trace, **(_trace_kwargs or {})
    )

    out = np.empty((B, S, HID), dtype=np.float32)
    for b in range(B):
        acc = res.results[b * 4]["out_p"].astype(np.float32)
        for hg in range(1, 4):
            acc = acc + res.results[b * 4 + hg]["out_p"].astype(np.float32)
        out[b] = acc
    if _trace:
        return out, res
    return out
